# revision 9
# baseline (speedup 1.0000x reference)
"""Causal self-attention (B=4, T=1024, C=1024, H=16) on 8 TRN2 NeuronCores.

Sharding: tensor-parallel over heads — 2 heads per core. x is replicated;
each core computes qkv for its heads, attention, and a partial output
projection (its heads' columns of w_out); the host sums the 8 partials.

v3: all matmul operands in bf16 (PSUM accumulation stays fp32); input DMAs
split across the sync and gpsimd queues; projection PSUM is DMA'd straight
to HBM in fp32 (no SBUF bounce copy); emission order per superstep is
attention(s) -> normalize(s) -> qkv(s+1) so the PE always has the next
group's qkv matmuls to chew on while the normalize chain runs; the qkT
copy and the softmax-denominator copies run on the scalar engine so they
overlap the DVE's reciprocal/normalize work.

Per-core dataflow (superstep s = token group tg = s = group (b, qg)):
  qkv:   qT/kT[chan, tok] = w.T @ x (one [128,2,512] PSUM tile);
         v[tok, chan] directly via x.T @ w_v per 128-token block.
  attn:  per 128-key block: ST[key, (head, query)] = kT.T @ qT (both heads
         in one [128,2,512] PSUM tile); PT = exp(ST/8) in ONE activation
         (bf16 out); diagonal 128x128 block masked multiplicatively on DVE;
         OT[d+1, query] += v_aug.T @ PT (row 64 accumulates the softmax
         denominator via the ones column of v_aug).
  norm:  y = OT[0:64] * bcast(1/OT[64]).
  proj:  out[tok, :] = yT.T @ woT per 128-token tile, both 512-col halves
         into one [128,1024] PSUM tile, DMA'd PSUM->HBM directly.
"""

import sys
import types

import numpy as np

import concourse.bacc as bacc
import concourse.mybir as mybir
import concourse.tile as tile
from concourse.bass_utils import run_bass_kernel_spmd

F32 = mybir.dt.float32
BF16 = mybir.dt.bfloat16
Exp = mybir.ActivationFunctionType.Exp

P = 128
B = 4
T = 1024
C = 1024
N_HEAD = 16
DH = 64
BT = B * T           # 4096 tokens
NCO = C // P         # 8 contraction blocks
NTG = BT // 512      # 8 token groups of 512
QG_PER_B = T // 512  # 2 query groups per batch
N_CORES = 8
H_LOC = N_HEAD // N_CORES  # 2 local heads

SCALE = 1.0 / np.sqrt(np.float32(DH))  # 0.125


def build_nc():
    nc = bacc.Bacc("TRN2", target_bir_lowering=False, debug=False)

    xr = nc.dram_tensor("xr", [P, NTG, NCO, 512], BF16, kind="ExternalInput")
    wr = nc.dram_tensor("wr", [P, NCO, 3 * P], BF16, kind="ExternalInput")
    wo = nc.dram_tensor("wo", [P, C], BF16, kind="ExternalInput")
    out = nc.dram_tensor("out", [BT, C], BF16, kind="ExternalOutput")

    with tile.TileContext(nc) as tc:
        with (
            tc.tile_pool(name="consts", bufs=1) as consts,
            tc.tile_pool(name="pt", bufs=4) as ptp,
            tc.tile_pool(name="outp", bufs=4) as outp,
            tc.tile_pool(name="ep", bufs=4) as epp,
            tc.tile_pool(name="psb", bufs=3, space="PSUM") as psb,
            tc.tile_pool(name="pso", bufs=1, space="PSUM") as pso,
        ):
            # ---- input staging ----
            # first token group + weights arrive co-chunk-interleaved across
            # two DMA queues so the PE can start as soon as chunk 0 lands
            x_sb = consts.tile([P, NTG, NCO, 512], BF16)
            w_sb = consts.tile([P, NCO, 3 * P], BF16)
            for co in range(NCO):
                nc.sync.dma_start(x_sb[:, 0, co, :], xr[:, 0, co, :])
                nc.gpsimd.dma_start(w_sb[:, co, :], wr[:, co, :])
            wo_sb = consts.tile([P, C], BF16)
            nc.gpsimd.dma_start(x_sb[:, 1, :, :], xr[:, 1, :, :])
            nc.gpsimd.dma_start(wo_sb[:], wo[:])
            for tg in range(2, NTG):
                nc.gpsimd.dma_start(x_sb[:, tg, :, :], xr[:, tg, :, :])

            # qkT[:, tg, 0, :] = qT, qkT[:, tg, 1, :] = kT  (chan, tok)
            qkT = consts.tile([P, NTG, 2, 512], BF16)
            # v with ones column at index DH (softmax denominator trick)
            v_aug = [
                consts.tile([P, BT // P, DH + 1], BF16, tag=f"v{h}", name=f"v{h}")
                for h in range(H_LOC)
            ]
            for h in range(H_LOC):
                nc.vector.memset(v_aug[h][:, :, DH], 1.0)

            # multiplicative causal mask for the diagonal 128x128 block of
            # PT[key, query]: keep where col - row >= 0
            tri01 = consts.tile([P, P], BF16)
            nc.gpsimd.memset(tri01[:], 1.0)
            nc.gpsimd.affine_select(
                out=tri01[:], in_=tri01[:],
                compare_op=mybir.AluOpType.is_ge, fill=0.0,
                base=0, pattern=[[1, P]], channel_multiplier=-1,
            )

            # ---- software-pipelined projection of the previous group ----
            proj_queue = []  # (tgq, yT, ti) steps awaiting emission
            ob_flip = [0]

            def proj_pair(tgq, yT, ti):
                tok0 = tgq * 512
                po = psb.tile([P, C], F32, tag="big", name="po")
                for half in range(2):
                    nc.tensor.matmul(
                        po[:, half * 512:(half + 1) * 512],
                        lhsT=yT[:, ti * P:(ti + 1) * P],
                        rhs=wo_sb[:, half * 512:(half + 1) * 512],
                        start=True, stop=True,
                    )
                ob = outp.tile([P, C], BF16, tag="ob", name="ob")
                if ob_flip[0] % 3 == 2:
                    nc.scalar.copy(ob[:], po[:])
                else:
                    nc.vector.tensor_copy(ob[:], po[:])
                ob_flip[0] += 1
                nc.sync.dma_start(
                    out[tok0 + ti * P:tok0 + (ti + 1) * P, :], ob[:]
                )

            def proj_slot():
                if proj_queue:
                    proj_pair(*proj_queue.pop(0))

            def emit_qkv(s):
                # qk projection for token group s
                qk_ps = psb.tile([P, 2, 512], F32, tag="big", name="qk_ps")
                for cb in range(2):  # 0=q, 1=k
                    for co in range(NCO):
                        nc.tensor.matmul(
                            qk_ps[:, cb, :],
                            lhsT=w_sb[:, co, cb * P:(cb + 1) * P],
                            rhs=x_sb[:, s, co, :],
                            start=(co == 0), stop=(co == NCO - 1),
                        )
                    proj_slot()
                # scalar engine: overlaps the DVE normalize of group s-1
                nc.scalar.copy(qkT[:, s, :, :], qk_ps[:])
                # v directly in [token, chan] layout: x.T @ w_v
                psv = psb.tile([P, 4, P], F32, tag="big", name="psv")
                for tb in range(4):
                    for co in range(NCO):
                        nc.tensor.matmul(
                            psv[:, tb, :],
                            lhsT=x_sb[:, s, co, tb * P:(tb + 1) * P],
                            rhs=w_sb[:, co, 2 * P:3 * P],
                            start=(co == 0), stop=(co == NCO - 1),
                        )
                proj_slot()
                for h in range(H_LOC):
                    nc.vector.tensor_copy(
                        v_aug[h][:, s * 4:(s + 1) * 4, 0:DH],
                        psv[:, :, h * DH:(h + 1) * DH],
                    )

            # ---- supersteps ----
            emit_qkv(0)
            for s in range(NTG):
                b, qg = divmod(s, QG_PER_B)

                # --- attention for group (b, qg) = superstep s ---
                nkj_total = (qg + 1) * 4
                slot_at = 1 if qg == 0 else 4  # pop proj late enough that
                # the previous group's yT (DVE) is certainly ready
                OT = pso.tile([DH + 1, 2, 512], F32, tag="ot", name="ot")
                pending = []  # [(pt, kb, q_lo, idx)] PV delayed 2 blocks

                def emit_pv(batch, nkj_total=nkj_total, OT=OT):
                    # per-head consecutive accumulation into the same PSUM
                    # bank pipelines much better than interleaved singles
                    for h in range(H_LOC):
                        for pt_, kb_, q_lo_, idx_ in batch:
                            nc.tensor.matmul(
                                OT[:, h, q_lo_:512],
                                lhsT=v_aug[h][:, kb_, :],
                                rhs=pt_[:, h, q_lo_:512],
                                start=(idx_ == 0),
                                stop=(idx_ == nkj_total - 1),
                            )

                idx = 0
                for kg in range(qg + 1):
                    diag = kg == qg
                    tgk = QG_PER_B * b + kg
                    for kj in range(4):
                        kb = tgk * 4 + kj
                        q_lo = kj * P if diag else 0
                        st = psb.tile([P, 2, 512], F32, tag="big", name="st")
                        for h in range(H_LOC):
                            hs = slice(h * DH, (h + 1) * DH)
                            nc.tensor.matmul(
                                st[:, h, q_lo:512],
                                lhsT=qkT[hs, tgk, 1, kj * P:(kj + 1) * P],
                                rhs=qkT[hs, s, 0, q_lo:512],
                                start=True, stop=True,
                            )
                        pt = ptp.tile([P, 2, 512], BF16, tag="pt", name="pt")
                        nc.scalar.activation(
                            pt[:, :, q_lo:512], st[:, :, q_lo:512], Exp,
                            bias=0.0, scale=float(SCALE),
                        )
                        if diag:
                            for h in range(H_LOC):
                                nc.vector.tensor_mul(
                                    pt[:, h, kj * P:(kj + 1) * P],
                                    pt[:, h, kj * P:(kj + 1) * P],
                                    tri01[:],
                                )
                        pending.append((pt, kb, q_lo, idx))
                        idx += 1
                        if len(pending) == 2 and idx < nkj_total:
                            emit_pv(pending)
                            pending = []
                        if idx >= slot_at:
                            proj_slot()
                emit_pv(pending)
                pending = []
                while proj_queue:
                    proj_slot()

                # --- epilogue: normalize y = O / den (den in OT row 64) ---
                last_group = s == NTG - 1
                yT = epp.tile([P, 512], BF16, tag="yt", name="yt")
                if not last_group:
                    den = epp.tile([1, 2, 512], F32, tag="den", name="den")
                    nc.vector.tensor_copy(den[:], OT[DH:DH + 1, :, :])
                    rc = epp.tile([1, 2, 512], F32, tag="rc", name="rc")
                    nc.vector.reciprocal_approx_fast(rc[:], den[:])
                    rbr = epp.tile([DH, 2, 512], F32, tag="rbr", name="rbr")
                    nc.gpsimd.partition_broadcast(rbr[:], rc[:], channels=DH)
                if not last_group:
                    for h in range(H_LOC):
                        nc.vector.tensor_mul(
                            yT[h * DH:(h + 1) * DH, :], OT[0:DH, h, :],
                            rbr[:, h, :],
                        )
                    # refill AFTER emitting qkv(s+1): the qkv-window slots
                    # must pop OLD pairs (whose yT is long ready), never
                    # pairs that depend on the normalize chain just emitted
                    emit_qkv(s + 1)
                    proj_queue = [(s, yT, ti) for ti in range(4)]
                else:
                    # tail: chunk the entire normalize per token tile and
                    # chase each chunk with its projection immediately
                    for ti in range(4):
                        ts_ = slice(ti * P, (ti + 1) * P)
                        den = epp.tile([1, 2, P], F32, tag="dent", name="den")
                        nc.vector.tensor_copy(den[:], OT[DH:DH + 1, :, ts_])
                        rc = epp.tile([1, 2, P], F32, tag="rct", name="rc")
                        nc.vector.reciprocal_approx_fast(rc[:], den[:])
                        rbr = epp.tile([DH, 2, P], F32, tag="rbrt", name="rbr")
                        nc.gpsimd.partition_broadcast(rbr[:], rc[:], channels=DH)
                        for h in range(H_LOC):
                            nc.vector.tensor_mul(
                                yT[h * DH:(h + 1) * DH, ts_],
                                OT[0:DH, h, ts_], rbr[:, h, :],
                            )
                        proj_pair(s, yT, ti)

            while proj_queue:
                proj_slot()

    nc.compile()
    return nc


def _install_ntff_hook():
    try:
        from antenv.axon_hooks import get_axon_ntff_profile_hook  # noqa: F401
        return
    except ImportError:
        pass
    try:
        import trn_agent_boot.trn_boot as tb
        hook = tb._ntff_profile_via_ctypes("/opt/axon/libaxon_pjrt.so")
        mod = types.ModuleType("antenv.axon_hooks")
        mod.get_axon_ntff_profile_hook = lambda: hook
        mod.set_axon_ntff_profile_hook = lambda h: None
        sys.modules["antenv.axon_hooks"] = mod
    except Exception:
        pass


_NC_CACHE = None
LAST_EXEC_NS = None
LAST_TRACE = None


def kernel(x, w_qkv, w_out, trace=False):
    global _NC_CACHE, LAST_EXEC_NS, LAST_TRACE
    import ml_dtypes
    BF = ml_dtypes.bfloat16

    if _NC_CACHE is None:
        _NC_CACHE = build_nc()
    nc = _NC_CACHE

    x = np.asarray(x, dtype=np.float32)
    w_qkv = np.asarray(w_qkv, dtype=np.float32)
    w_out = np.asarray(w_out, dtype=np.float32)

    # x host layout [p, tg, co, t]: x[chan co*128+p, tok tg*512+t]
    xf = x.reshape(BT, C)
    xr_np = np.ascontiguousarray(
        xf.reshape(NTG, 512, NCO, P).transpose(3, 0, 2, 1).astype(BF)
    )
    in_maps = []
    for core in range(N_CORES):
        h0 = core * H_LOC * DH  # first local channel
        ch = slice(h0, h0 + H_LOC * DH)
        # wq columns: [q chans | k chans | v chans] for the local heads
        wq_i = np.concatenate(
            [w_qkv[s * C:s * C + C, :][ch, :] for s in range(3)], axis=0
        )  # [384, C]
        wr_np = np.ascontiguousarray(
            wq_i.T.reshape(NCO, P, 3 * P).transpose(1, 0, 2).astype(BF)
        )  # [128, 8, 384]
        in_maps.append({
            "xr": xr_np,
            "wr": wr_np,
            "wo": np.ascontiguousarray(w_out[:, ch].T.astype(BF)),  # [128, C]
        })

    if trace:
        _install_ntff_hook()
    res = run_bass_kernel_spmd(
        nc, in_maps, core_ids=list(range(N_CORES)), trace=trace
    )
    LAST_EXEC_NS = res.exec_time_ns
    kernel_globals = globals()
    kernel_globals['LAST_RESULT'] = res
    LAST_TRACE = (
        res.instructions_and_trace[1] if res.instructions_and_trace else None
    )

    acc = np.zeros((BT, C), dtype=np.float64)
    for core in range(N_CORES):
        acc += res.results[core]["out"].astype(np.float64)
    return acc.astype(np.float32).reshape(B, T, C)


# revision 10
# speedup vs baseline: 1.2131x; 1.2131x over previous
"""Causal self-attention (B=4, T=1024, C=1024, H=16) on 8 TRN2 NeuronCores.

Sharding: tensor-parallel over heads — 2 heads per core. x is replicated;
each core computes qkv for its heads, attention, and a partial output
projection (its heads' columns of w_out); the host sums the 8 partials.

v3: all matmul operands in bf16 (PSUM accumulation stays fp32); input DMAs
split across the sync and gpsimd queues; projection PSUM is DMA'd straight
to HBM in fp32 (no SBUF bounce copy); emission order per superstep is
attention(s) -> normalize(s) -> qkv(s+1) so the PE always has the next
group's qkv matmuls to chew on while the normalize chain runs; the qkT
copy and the softmax-denominator copies run on the scalar engine so they
overlap the DVE's reciprocal/normalize work.

Per-core dataflow (superstep s = token group tg = s = group (b, qg)):
  qkv:   qT/kT[chan, tok] = w.T @ x (one [128,2,512] PSUM tile);
         v[tok, chan] directly via x.T @ w_v per 128-token block.
  attn:  per 128-key block: ST[key, (head, query)] = kT.T @ qT (both heads
         in one [128,2,512] PSUM tile); PT = exp(ST/8) in ONE activation
         (bf16 out); diagonal 128x128 block masked multiplicatively on DVE;
         OT[d+1, query] += v_aug.T @ PT (row 64 accumulates the softmax
         denominator via the ones column of v_aug).
  norm:  y = OT[0:64] * bcast(1/OT[64]).
  proj:  out[tok, :] = yT.T @ woT per 128-token tile, both 512-col halves
         into one [128,1024] PSUM tile, DMA'd PSUM->HBM directly.
"""

import sys
import types

import numpy as np

import concourse.bacc as bacc
import concourse.mybir as mybir
import concourse.tile as tile
from concourse.bass_utils import run_bass_kernel_spmd

F32 = mybir.dt.float32
BF16 = mybir.dt.bfloat16
Exp = mybir.ActivationFunctionType.Exp

P = 128
B = 4
T = 1024
C = 1024
N_HEAD = 16
DH = 64
BT = B * T           # 4096 tokens
NCO = C // P         # 8 contraction blocks
NTG = BT // 512      # 8 token groups of 512
QG_PER_B = T // 512  # 2 query groups per batch
N_CORES = 8
H_LOC = N_HEAD // N_CORES  # 2 local heads

SCALE = 1.0 / np.sqrt(np.float32(DH))  # 0.125


def build_nc():
    nc = bacc.Bacc("TRN2", target_bir_lowering=False, debug=False)

    xr = nc.dram_tensor("xr", [P, NTG, NCO, 512], BF16, kind="ExternalInput")
    wr = nc.dram_tensor("wr", [P, NCO, 3 * P], BF16, kind="ExternalInput")
    wo = nc.dram_tensor("wo", [P, C], BF16, kind="ExternalInput")
    out = nc.dram_tensor("out", [BT, C], BF16, kind="ExternalOutput")

    with tile.TileContext(nc) as tc:
        with (
            tc.tile_pool(name="consts", bufs=1) as consts,
            tc.tile_pool(name="pt", bufs=5) as ptp,
            tc.tile_pool(name="outp", bufs=4) as outp,
            tc.tile_pool(name="ep", bufs=4) as epp,
            tc.tile_pool(name="psb", bufs=3, space="PSUM") as psb,
            tc.tile_pool(name="pso", bufs=1, space="PSUM") as pso,
        ):
            # ---- input staging ----
            # first token group + weights arrive co-chunk-interleaved across
            # two DMA queues so the PE can start as soon as chunk 0 lands
            x_sb = consts.tile([P, NTG, NCO, 512], BF16)
            w_sb = consts.tile([P, NCO, 3 * P], BF16)
            for co in range(NCO):
                nc.sync.dma_start(x_sb[:, 0, co, :], xr[:, 0, co, :])
                nc.gpsimd.dma_start(w_sb[:, co, :], wr[:, co, :])
            wo_sb = consts.tile([P, C], BF16)
            nc.gpsimd.dma_start(x_sb[:, 1, :, :], xr[:, 1, :, :])
            nc.gpsimd.dma_start(wo_sb[:], wo[:])
            for tg in range(2, NTG):
                nc.gpsimd.dma_start(x_sb[:, tg, :, :], xr[:, tg, :, :])

            # qkT[:, tg, 0, :] = qT, qkT[:, tg, 1, :] = kT  (chan, tok)
            qkT = consts.tile([P, NTG, 2, 512], BF16)
            # v with ones column at index DH (softmax denominator trick)
            v_aug = [
                consts.tile([P, BT // P, DH + 1], BF16, tag=f"v{h}", name=f"v{h}")
                for h in range(H_LOC)
            ]
            for h in range(H_LOC):
                nc.vector.memset(v_aug[h][:, :, DH], 1.0)

            # multiplicative causal mask for the diagonal 128x128 block of
            # PT[key, query]: keep where col - row >= 0
            tri01 = consts.tile([P, P], BF16)
            nc.gpsimd.memset(tri01[:], 1.0)
            nc.gpsimd.affine_select(
                out=tri01[:], in_=tri01[:],
                compare_op=mybir.AluOpType.is_ge, fill=0.0,
                base=0, pattern=[[1, P]], channel_multiplier=-1,
            )

            # ---- software-pipelined projection of the previous group ----
            proj_queue = []  # (tgq, yT, ti) steps awaiting emission
            ob_flip = [0]

            def proj_pair(tgq, yT, ti):
                tok0 = tgq * 512
                po = psb.tile([P, C], F32, tag="big", name="po")
                for half in range(2):
                    nc.tensor.matmul(
                        po[:, half * 512:(half + 1) * 512],
                        lhsT=yT[:, ti * P:(ti + 1) * P],
                        rhs=wo_sb[:, half * 512:(half + 1) * 512],
                        start=True, stop=True,
                    )
                ob = outp.tile([P, C], BF16, tag="ob", name="ob")
                if ob_flip[0] % 3 == 2:
                    nc.scalar.copy(ob[:], po[:])
                else:
                    nc.vector.tensor_copy(ob[:], po[:])
                ob_flip[0] += 1
                nc.sync.dma_start(
                    out[tok0 + ti * P:tok0 + (ti + 1) * P, :], ob[:]
                )

            def proj_slot():
                if proj_queue:
                    proj_pair(*proj_queue.pop(0))

            def emit_qkv(s):
                # qk projection for token group s
                qk_ps = psb.tile([P, 2, 512], F32, tag="big", name="qk_ps")
                for cb in range(2):  # 0=q, 1=k
                    for co in range(NCO):
                        nc.tensor.matmul(
                            qk_ps[:, cb, :],
                            lhsT=w_sb[:, co, cb * P:(cb + 1) * P],
                            rhs=x_sb[:, s, co, :],
                            start=(co == 0), stop=(co == NCO - 1),
                        )
                    proj_slot()
                # scalar engine: overlaps the DVE normalize of group s-1
                nc.scalar.copy(qkT[:, s, :, :], qk_ps[:])
                # v directly in [token, chan] layout: x.T @ w_v
                psv = psb.tile([P, 4, P], F32, tag="big", name="psv")
                for tb in range(4):
                    for co in range(NCO):
                        nc.tensor.matmul(
                            psv[:, tb, :],
                            lhsT=x_sb[:, s, co, tb * P:(tb + 1) * P],
                            rhs=w_sb[:, co, 2 * P:3 * P],
                            start=(co == 0), stop=(co == NCO - 1),
                        )
                proj_slot()
                for h in range(H_LOC):
                    nc.vector.tensor_copy(
                        v_aug[h][:, s * 4:(s + 1) * 4, 0:DH],
                        psv[:, :, h * DH:(h + 1) * DH],
                    )

            # ---- supersteps ----
            emit_qkv(0)
            for s in range(NTG):
                b, qg = divmod(s, QG_PER_B)

                # --- attention for group (b, qg) = superstep s ---
                nkj_total = (qg + 1) * 4
                slot_at = 1 if qg == 0 else 4  # pop proj late enough that
                # the previous group's yT (DVE) is certainly ready
                OT = pso.tile([DH + 1, 2, 512], F32, tag="ot", name="ot")
                pending = []  # [(pt, kb, q_lo, idx)] PV delayed 2-3 blocks
                first_emitted = [False]

                def emit_pv(batch, nkj_total=nkj_total, OT=OT):
                    # per-head consecutive accumulation into the same PSUM
                    # bank pipelines much better than interleaved singles
                    for h in range(H_LOC):
                        for pt_, kb_, q_lo_, idx_ in batch:
                            nc.tensor.matmul(
                                OT[:, h, q_lo_:512],
                                lhsT=v_aug[h][:, kb_, :],
                                rhs=pt_[:, h, q_lo_:512],
                                start=(idx_ == 0),
                                stop=(idx_ == nkj_total - 1),
                            )

                idx = 0
                for kg in range(qg + 1):
                    diag = kg == qg
                    tgk = QG_PER_B * b + kg
                    for kj in range(4):
                        kb = tgk * 4 + kj
                        q_lo = kj * P if diag else 0
                        st = psb.tile([P, 2, 512], F32, tag="big", name="st")
                        for h in range(H_LOC):
                            hs = slice(h * DH, (h + 1) * DH)
                            nc.tensor.matmul(
                                st[:, h, q_lo:512],
                                lhsT=qkT[hs, tgk, 1, kj * P:(kj + 1) * P],
                                rhs=qkT[hs, s, 0, q_lo:512],
                                start=True, stop=True,
                            )
                        pt = ptp.tile([P, 2, 512], BF16, tag="pt", name="pt")
                        nc.scalar.activation(
                            pt[:, :, q_lo:512], st[:, :, q_lo:512], Exp,
                            bias=0.0, scale=float(SCALE),
                        )
                        if diag:
                            for h in range(H_LOC):
                                nc.vector.tensor_mul(
                                    pt[:, h, kj * P:(kj + 1) * P],
                                    pt[:, h, kj * P:(kj + 1) * P],
                                    tri01[:],
                                )
                        pending.append((pt, kb, q_lo, idx))
                        idx += 1
                        need = 2 if first_emitted[0] else 3
                        if len(pending) == need and idx < nkj_total:
                            emit_pv(pending)
                            pending = []
                            first_emitted[0] = True
                        if idx >= slot_at:
                            proj_slot()
                emit_pv(pending)
                pending = []
                while proj_queue:
                    proj_slot()

                # --- epilogue: normalize y = O / den (den in OT row 64) ---
                last_group = s == NTG - 1
                yT = epp.tile([P, 512], BF16, tag="yt", name="yt")
                if not last_group:
                    den = epp.tile([1, 2, 512], F32, tag="den", name="den")
                    nc.scalar.copy(den[:], OT[DH:DH + 1, :, :])
                    rc = epp.tile([1, 2, 512], F32, tag="rc", name="rc")
                    nc.vector.reciprocal_approx_fast(rc[:], den[:])
                    rbr = epp.tile([DH, 2, 512], F32, tag="rbr", name="rbr")
                    nc.gpsimd.partition_broadcast(rbr[:], rc[:], channels=DH)
                if not last_group:
                    for h in range(H_LOC):
                        nc.vector.tensor_mul(
                            yT[h * DH:(h + 1) * DH, :], OT[0:DH, h, :],
                            rbr[:, h, :],
                        )
                    # refill AFTER emitting qkv(s+1): the qkv-window slots
                    # must pop OLD pairs (whose yT is long ready), never
                    # pairs that depend on the normalize chain just emitted
                    emit_qkv(s + 1)
                    proj_queue = [(s, yT, ti) for ti in range(4)]
                else:
                    # tail: one merged den/recip/bcast, then per-token-tile
                    # normalize chased immediately by its projection
                    den = epp.tile([1, 2, 512], F32, tag="den", name="den")
                    nc.scalar.copy(den[:], OT[DH:DH + 1, :, :])
                    rc = epp.tile([1, 2, 512], F32, tag="rc", name="rc")
                    nc.vector.reciprocal_approx_fast(rc[:], den[:])
                    rbr = epp.tile([DH, 2, 512], F32, tag="rbr", name="rbr")
                    nc.gpsimd.partition_broadcast(rbr[:], rc[:], channels=DH)
                    for ti in range(4):
                        ts_ = slice(ti * P, (ti + 1) * P)
                        for h in range(H_LOC):
                            nc.vector.tensor_mul(
                                yT[h * DH:(h + 1) * DH, ts_],
                                OT[0:DH, h, ts_], rbr[:, h, ts_],
                            )
                        proj_pair(s, yT, ti)

            while proj_queue:
                proj_slot()

    nc.compile()
    return nc


def _install_ntff_hook():
    try:
        from antenv.axon_hooks import get_axon_ntff_profile_hook  # noqa: F401
        return
    except ImportError:
        pass
    try:
        import trn_agent_boot.trn_boot as tb
        hook = tb._ntff_profile_via_ctypes("/opt/axon/libaxon_pjrt.so")
        mod = types.ModuleType("antenv.axon_hooks")
        mod.get_axon_ntff_profile_hook = lambda: hook
        mod.set_axon_ntff_profile_hook = lambda h: None
        sys.modules["antenv.axon_hooks"] = mod
    except Exception:
        pass


_NC_CACHE = None
LAST_EXEC_NS = None
LAST_TRACE = None


def kernel(x, w_qkv, w_out, trace=False):
    global _NC_CACHE, LAST_EXEC_NS, LAST_TRACE
    import ml_dtypes
    BF = ml_dtypes.bfloat16

    if _NC_CACHE is None:
        _NC_CACHE = build_nc()
    nc = _NC_CACHE

    x = np.asarray(x, dtype=np.float32)
    w_qkv = np.asarray(w_qkv, dtype=np.float32)
    w_out = np.asarray(w_out, dtype=np.float32)

    # x host layout [p, tg, co, t]: x[chan co*128+p, tok tg*512+t]
    xf = x.reshape(BT, C)
    xr_np = np.ascontiguousarray(
        xf.reshape(NTG, 512, NCO, P).transpose(3, 0, 2, 1).astype(BF)
    )
    in_maps = []
    for core in range(N_CORES):
        h0 = core * H_LOC * DH  # first local channel
        ch = slice(h0, h0 + H_LOC * DH)
        # wq columns: [q chans | k chans | v chans] for the local heads
        wq_i = np.concatenate(
            [w_qkv[s * C:s * C + C, :][ch, :] for s in range(3)], axis=0
        )  # [384, C]
        wr_np = np.ascontiguousarray(
            wq_i.T.reshape(NCO, P, 3 * P).transpose(1, 0, 2).astype(BF)
        )  # [128, 8, 384]
        in_maps.append({
            "xr": xr_np,
            "wr": wr_np,
            "wo": np.ascontiguousarray(w_out[:, ch].T.astype(BF)),  # [128, C]
        })

    if trace:
        _install_ntff_hook()
    res = run_bass_kernel_spmd(
        nc, in_maps, core_ids=list(range(N_CORES)), trace=trace
    )
    LAST_EXEC_NS = res.exec_time_ns
    kernel_globals = globals()
    kernel_globals['LAST_RESULT'] = res
    LAST_TRACE = (
        res.instructions_and_trace[1] if res.instructions_and_trace else None
    )

    acc = np.zeros((BT, C), dtype=np.float64)
    for core in range(N_CORES):
        acc += res.results[core]["out"].astype(np.float64)
    return acc.astype(np.float32).reshape(B, T, C)
